# revision 15
# baseline (speedup 1.0000x reference)
"""Anisotropic collisions kernel for 8 TRN2 NeuronCores.

Math: for each of 9*64*64 = 36864 independent systems (mode, spatial cell),
build tridiagonal coefficients from Rosenbluth cumulative integrals of
flm(v) along v (512 points), then solve the tridiagonal system along v.

Reformulation (validated numerically, rel err ~5e-6 vs f64 Thomas):
  G1 = cumsum(y*g1(v)) + 2*S1,  g1 = 3v^2 - v^4 - 2v
  G2 = cumsum(y*g2(v)) + S1,    g2 = v^4 - v
  S1 = sum(y*v)
  w = G1*KY/(2*DV*v^3); u = G2*KY/(DV^2*v^2); KY = 4*pi*Y_DT/3
  a = u - w; c = u + w; b = 1 + 8*pi*Y_DT*y + u/2 - il2*(2*DV/v)*w
Solve via scan-form Thomas: cp ~= c/(b - a*shift(c/b)) (one fixed-point
refinement of the continued fraction -- strongly diagonally dominant since
Y_DT=1e-12), then dp and back-substitution are first-order linear
recurrences computed with tensor_tensor_scan.

Layout: batch on partitions, v along free. Each SBUF tile [128, 4*512]
holds 512 systems (4 consecutive shard rows per partition). Scan "resets"
at system boundaries come from zeros in the scan multiplier column (a[v=0]
and cp[v=511] are unused by Thomas, so zeroing them is exact).

Toolchain notes: this walrus build accepts only ONE sync-wait per
instruction and rejects custom-DVE InstISA ops, so we use standard ISA ops
only and split multi-wait instructions into standalone InstEventSemaphore
waits in a post-pass.
"""

import numpy as np
from contextlib import ExitStack

import concourse.bass as bass
import concourse.tile as tile
import concourse.mybir as mybir
from concourse.bass_utils import run_bass_kernel_spmd

F32 = mybir.dt.float32

NX, NY, NV = 64, 64, 512
N_MODES = 9
DV = 0.015625
Y_DT = 1.0e-12
FOUR_PI = 4.0 * np.pi
KY = FOUR_PI * Y_DT / 3.0

N_CORES = 8
ROWS_TOTAL = N_MODES * NX * NY            # 36864
ROWS_PER_CORE = ROWS_TOTAL // N_CORES     # 4608
FUSE = 4                                  # systems per partition row
GROUP_ROWS = 128 * FUSE                   # 512 systems per group
N_GROUPS = ROWS_PER_CORE // GROUP_ROWS    # 9
FD = FUSE * NV                            # 2048

REFINE = False                            # one cp fixed-point refinement

_V = (np.arange(NV, dtype=np.float64) + 1.0) * DV

# constant profile blob layout (each [128, FD] f32, rows replicated):
_C_NAMES = ["resetv", "reset1", "g1w", "g2w", "t1c", "pw2k", "pu2k"]
NC_CONST = len(_C_NAMES)


def _profiles():
    v = _V
    g1w = 3.0 * v**2 - v**4 - 2.0 * v
    g2w = v**4 - v
    t1c = -2.0 * DV / v                   # t1 = wn*t1c = +coeff1*Y_DT/v
    pwn = -KY / (2.0 * DV * v**3)         # wn = (G1b + 2 S1)*pwn = -w
    pun = -KY / (DV * DV * v**2)          # un = (G2b + S1)*pun = -u
    r1 = np.ones(NV)
    r1[1:] = v[:-1] / v[1:]
    r1[0] = 0.0                           # scan reset at each system start
    ones0 = np.ones(NV)
    ones0[0] = 0.0
    prof = {
        "g1w": g1w, "g2w": g2w, "t1c": t1c,
        "resetv": r1, "reset1": ones0, "pw2k": pwn, "pu2k": pun,
    }
    return np.concatenate([np.tile(prof[n], FUSE) for n in _C_NAMES])


def _legalize_multiwait(nc):
    """Split instructions with >1 sync wait: keep one wait on the
    instruction, hoist the rest onto standalone InstEventSemaphore ops
    immediately before it on the same engine (this walrus accepts only one
    wait per instruction)."""
    n = [0]

    def fresh(engine, wait):
        n[0] += 1
        return mybir.InstEventSemaphore(
            name=f"mwsplit-{n[0]}",
            engine=engine,
            sync_info=mybir.SyncInfo(on_wait=[wait], on_update=[]),
        )

    for fn in nc.m.functions:
        for blk in fn.blocks:
            out = []
            for ins in blk.instructions:
                si = ins.sync_info
                if si is not None and si.on_wait is not None and len(si.on_wait) > 1:
                    waits = list(si.on_wait)
                    for w in waits[:-1]:
                        out.append(fresh(ins.engine, w))
                    si.on_wait = [waits[-1]]
                out.append(ins)
            blk.instructions[:] = out


def build_nc(n_groups=N_GROUPS, legalize=True, repeat=1):
    nc = bass.Bass()
    rows = n_groups * GROUP_ROWS
    y_in = nc.declare_dram_parameter("y", [rows, NV], F32, isOutput=False)
    il2_in = nc.declare_dram_parameter("il2", [128, n_groups], F32, isOutput=False)
    cst_in = nc.declare_dram_parameter("cst", [128, NC_CONST * FD], F32, isOutput=False)
    out_ext = nc.declare_dram_parameter("out", [rows, NV], F32, isOutput=True)

    MUL = mybir.AluOpType.mult
    ADD = mybir.AluOpType.add
    SUB = mybir.AluOpType.subtract
    COPY = mybir.ActivationFunctionType.Copy

    with ExitStack() as ctx:
        tc = ctx.enter_context(tile.TileContext(nc))
        cpool = ctx.enter_context(tc.tile_pool(name="consts", bufs=1))

        cst = cpool.tile([128, NC_CONST * FD], F32, tag="cst")
        H = 4 * FD
        nc.gpsimd.dma_start(cst[:, 0:H], cst_in[:, 0:H])
        nc.gpsimd.dma_start(cst[:, H:NC_CONST * FD], cst_in[:, H:NC_CONST * FD])
        C = {nm: cst[:, i * FD:(i + 1) * FD] for i, nm in enumerate(_C_NAMES)}
        touch_a = cpool.tile([128, 1], F32, tag="touch_a")
        nc.vector.tensor_copy(out=touch_a[:, :], in_=cst[:, 0:1])
        touch_c = cpool.tile([128, 1], F32, tag="touch_c")
        nc.vector.tensor_copy(out=touch_c[:, :], in_=cst[:, H:H + 1])
        io = ctx.enter_context(tc.tile_pool(name="io", bufs=2))
        wk = ctx.enter_context(tc.tile_pool(name="work", bufs=1))
        il2t = cpool.tile([128, n_groups], F32, tag="il2")
        nc.gpsimd.dma_start(il2t[:, :], il2_in[:, :])
        touch_b = cpool.tile([128, 1], F32, tag="touch_b")
        nc.vector.tensor_copy(out=touch_b[:, :], in_=il2t[:, 0:1])

        for rep in range(repeat):
          for g in range(n_groups):
            rsl = slice(g * GROUP_ROWS, (g + 1) * GROUP_ROWS)
            y_src = y_in[rsl, :].rearrange("(p j) v -> p (j v)", p=128)
            x_dst = out_ext[rsl, :].rearrange("(p j) v -> p (j v)", p=128)

            y4 = io.tile([128, FD], F32, tag="y4")
            nc.gpsimd.dma_start(y4[:, :], y_src)

            # t3 = 1 + 8*pi*Y_DT*y   (ACT)
            t3 = io.tile([128, FD], F32, tag="t3")
            nc.scalar.activation(t3[:, :], y4[:, :], COPY,
                                 bias=1.0, scale=float(8.0 * np.pi * Y_DT))

            wg1 = wk.tile([128, FD], F32, tag="T1")
            nc.vector.tensor_tensor(out=wg1[:, :], in0=y4[:, :], in1=C["g1w"], op=MUL)
            wg2 = wk.tile([128, FD], F32, tag="T2")
            nc.vector.tensor_tensor(out=wg2[:, :], in0=y4[:, :], in1=C["g2w"], op=MUL)

            # E1 = C1/v per system (ratio scan); S1 = E1[v_last] * v_last
            E1 = wk.tile([128, FD], F32, tag="T3")
            nc.vector.tensor_tensor_scan(E1[:, :], C["resetv"], y4[:, :], 0.0,
                                         op0=MUL, op1=ADD)
            s1x = wk.tile([128, FUSE], F32, tag="s1x")
            nc.scalar.activation(s1x[:, :], E1[:, NV - 1::NV], COPY,
                                 bias=0.0, scale=float(_V[-1]))
            s1x2 = wk.tile([128, FUSE], F32, tag="s1x2")
            nc.scalar.activation(s1x2[:, :], s1x[:, :], COPY, bias=0.0, scale=2.0)

            # Inject the S1 terms into the first column of each system's
            # weighted input: the cumsum then carries G1b+2*S1 / G2b+S1.
            nc.vector.tensor_tensor(out=wg1[:, 0::NV], in0=wg1[:, 0::NV],
                                    in1=s1x2[:, :], op=ADD)
            nc.vector.tensor_tensor(out=wg2[:, 0::NV], in0=wg2[:, 0::NV],
                                    in1=s1x[:, :], op=ADD)
            G1b = wk.tile([128, FD], F32, tag="T4")
            nc.vector.tensor_tensor_scan(G1b[:, :], C["reset1"], wg1[:, :], 0.0,
                                         op0=MUL, op1=ADD)
            G2b = wk.tile([128, FD], F32, tag="T5")
            nc.vector.tensor_tensor_scan(G2b[:, :], C["reset1"], wg2[:, :], 0.0,
                                         op0=MUL, op1=ADD)
            wn = wk.tile([128, FD], F32, tag="T3")
            nc.vector.tensor_tensor(out=wn[:, :], in0=G1b[:, :], in1=C["pw2k"], op=MUL)
            un = wk.tile([128, FD], F32, tag="T6")
            nc.vector.tensor_tensor(out=un[:, :], in0=G2b[:, :], in1=C["pu2k"], op=MUL)

            a_pos = wk.tile([128, FD], F32, tag="T1b")   # a = u - w
            nc.vector.tensor_tensor(out=a_pos[:, :], in0=wn[:, :], in1=un[:, :], op=SUB)
            c_pos = wk.tile([128, FD], F32, tag="T2b")   # c = u + w
            nc.vector.scalar_tensor_tensor(out=c_pos[:, :], in0=un[:, :], scalar=-1.0,
                                           in1=wn[:, :], op0=MUL, op1=SUB)
            t1 = wk.tile([128, FD], F32, tag="T1")      # +coeff1*Y/v
            nc.vector.tensor_tensor(out=t1[:, :], in0=wn[:, :], in1=C["t1c"], op=MUL)
            b1 = wk.tile([128, FD], F32, tag="T2")      # t3 + u/2
            nc.vector.scalar_tensor_tensor(out=b1[:, :], in0=un[:, :], scalar=-0.5,
                                           in1=t3[:, :], op0=MUL, op1=ADD)
            bn = wk.tile([128, FD], F32, tag="T5")      # -b
            nc.vector.scalar_tensor_tensor(out=bn[:, :], in0=t1[:, :],
                                           scalar=il2t[:, g:g + 1],
                                           in1=b1[:, :], op0=MUL, op1=SUB)
            binv_n = wk.tile([128, FD], F32, tag="T8")  # -1/b
            nc.vector.reciprocal(out=binv_n[:, :], in_=bn[:, :])

            if REFINE:
                # den = b - a*shift(c/b); dinv_n = -1/den
                mcp0g = wk.tile([128, FD + 1], F32, tag="T9")
                nc.vector.memset(mcp0g[:, 0:1], 0.0)
                nc.vector.tensor_tensor(out=mcp0g[:, 1:FD + 1], in0=c_pos[:, :],
                                        in1=binv_n[:, :], op=MUL)  # -cp0
                tpp = wk.tile([128, FD], F32, tag="T1")
                nc.vector.tensor_tensor(out=tpp[:, :], in0=a_pos[:, :],
                                        in1=mcp0g[:, 0:FD], op=MUL)  # -a*cp0sh
                tppv = tpp[:, :].rearrange("p (j v) -> p j v", j=FUSE)
                nc.vector.memset(tppv[:, :, 0:1], 0.0)
                den_n = wk.tile([128, FD], F32, tag="T2")
                nc.vector.tensor_tensor(out=den_n[:, :], in0=bn[:, :],
                                        in1=tpp[:, :], op=SUB)  # -den
                dinv_n = wk.tile([128, FD], F32, tag="T8")
                nc.vector.reciprocal(out=dinv_n[:, :], in_=den_n[:, :])  # -1/den
            else:
                dinv_n = binv_n

            alpha = wk.tile([128, FD], F32, tag="T1")   # -a/den
            nc.vector.tensor_tensor(out=alpha[:, :], in0=a_pos[:, :],
                                    in1=dinv_n[:, :], op=MUL)
            av = alpha[:, :].rearrange("p (j v) -> p j v", j=FUSE)
            nc.vector.memset(av[:, :, 0:1], 0.0)        # scan reset at v=0
            beta = wk.tile([128, FD], F32, tag="T5")    # +d/den
            nc.vector.scalar_tensor_tensor(out=beta[:, :], in0=dinv_n[:, :],
                                           scalar=-1.0, in1=y4[:, :],
                                           op0=MUL, op1=MUL)
            mcp = wk.tile([128, FD], F32, tag="T2")     # -c/den
            nc.vector.tensor_tensor(out=mcp[:, :], in0=c_pos[:, :],
                                    in1=dinv_n[:, :], op=MUL)
            mv = mcp[:, :].rearrange("p (j v) -> p j v", j=FUSE)
            nc.vector.memset(mv[:, :, NV - 1:NV], 0.0)  # bwd scan reset at v=511

            dp = wk.tile([128, FD], F32, tag="T10")
            nc.vector.tensor_tensor_scan(dp[:, :], alpha[:, :], beta[:, :], 0.0,
                                         op0=MUL, op1=ADD)
            x4 = io.tile([128, FD], F32, tag="x4")
            nc.vector.tensor_tensor_scan(x4[:, ::-1], mcp[:, ::-1], dp[:, ::-1], 0.0,
                                         op0=MUL, op1=ADD)
            nc.gpsimd.dma_start(x_dst, x4[:, :])

    if legalize:
        _legalize_multiwait(nc)
    return nc


_NC_CACHE = {}


def _get_nc(n_groups=N_GROUPS):
    if n_groups not in _NC_CACHE:
        _NC_CACHE[n_groups] = build_nc(n_groups)
    return _NC_CACHE[n_groups]


def make_inputs(y_shard, il2_rows, n_groups=N_GROUPS):
    """Per-core input map. y_shard [rows, 512] f32; il2_rows [rows] f32."""
    cst = np.broadcast_to(_profiles()[None, :], (128, NC_CONST * FD)
                          ).astype(np.float32).copy()
    il2 = il2_rows.reshape(n_groups, 128, FUSE)[:, :, 0].T.astype(np.float32).copy()
    return {
        "y": np.ascontiguousarray(y_shard, dtype=np.float32),
        "il2": il2,
        "cst": cst,
    }


def kernel(y, il_arr):
    y = np.asarray(y, dtype=np.float32)
    il_arr = np.asarray(il_arr)
    yf = y.reshape(ROWS_TOTAL, NV)
    il_f = il_arr.astype(np.float64)
    il2_all = np.repeat(il_f * (il_f + 1.0) / 2.0, NX * NY).astype(np.float32)

    nc = _get_nc()
    in_maps = []
    for c in range(N_CORES):
        rs = slice(c * ROWS_PER_CORE, (c + 1) * ROWS_PER_CORE)
        in_maps.append(make_inputs(yf[rs], il2_all[rs]))
    res = run_bass_kernel_spmd(nc, in_maps, core_ids=list(range(N_CORES)))
    outs = [res.results[c]["out"] for c in range(N_CORES)]
    x = np.concatenate(outs, axis=0).reshape(N_MODES, NX, NY, NV)
    return x.astype(np.float32)
